# revision 13
# baseline (speedup 1.0000x reference)
"""GCN VGAE encoder (nn_Encoder_25065429139538) on 8 Trainium2 NeuronCores.

Strategy (sharding_hint: shard nodes across cores, partition edges by dst,
replicate weights):
  - Nodes padded to 102400 = 8 x 12800; core d owns dst rows [d*SH, (d+1)*SH).
  - GCN norm dinv[src]*ew*dinv[dst] is factored: dinv[src] is pre-multiplied
    into the stored gather-table rows, dinv[dst] is applied when evicting the
    per-block accumulator from PSUM. Layers 1/2 then carry only the raw edge
    weight ew per edge; the mu/logstd layer needs no per-edge scalar at all
    (duplicate (src,dst) pairs stay as distinct slots, reproducing the
    reference's multiplicity semantics).
  - Tables are f16 [rows, 128] (features 0..63 real, 64..127 zero) so each
    dma_gather descriptor is the 256B minimum. Edges are bucketed by
    (src quarter q, dst 128-block t); each (q,t) cell is padded to whole
    128-slot chunks shared across cores. Per chunk, a one-hot matrix
    B[slot, dloc] (f16, built on DVE from cached dloc bytes) is the
    *stationary* matmul operand (128 cols -> fast weight load) and the
    gathered rows stream through, accumulating agg[dst, F] in PSUM.
  - Self-loops are an identity-weight matmul reading the core's own
    projected rows straight from SBUF (q0 phase, before any AllGather).
  - Per-edge metadata (gather indices, dloc, ew) is SBUF-resident for the
    whole kernel, loaded once in a few large DMAs.
  - h is AllGather'd between layers in 4 quarter-collectives, each fired as
    soon as its quarter of the projection completes, so the next layer's
    gathers overlap the exchange. mu/logstd share one aggregation pass.
"""

import math

import numpy as np

import concourse.bass as bass
import concourse.bacc as bacc
import concourse.mybir as mybir
import concourse.tile as tile
from concourse.bass_utils import run_bass_kernel_spmd
from concourse.library_config import mlp

# ---- problem constants (hardcoded per contract) ----
N = 100000
FIN, HID, OUT = 128, 64, 32
NCORES = 8

# ---- layout constants ----
SH = 12800            # rows per core
NPAD = SH * NCORES    # 102400
NBLK = SH // 128      # 100 dst blocks per core
NQ = 4                # src quarters
QBLK = NBLK // NQ     # 25 dst blocks per quarter
QROWS = SH // NQ      # 3200 rows per quarter per core
SUBROWS = QROWS * NCORES  # 25600 rows per quarter subtable
TF = 128              # padded table feature dim (f16 -> 256B rows)
SLOTS = 1024          # gather slots per dma_gather instruction
CPG = SLOTS // 128    # chunks per gather group = 32
SENT = 255.0          # dloc sentinel for padding slots


def _prep(edge_index, edge_weight):
    """Host-side edge partitioning."""
    src = np.asarray(edge_index[0], dtype=np.int64)
    dst = np.asarray(edge_index[1], dtype=np.int64)
    ew = np.asarray(edge_weight, dtype=np.float32)

    deg_w = np.zeros(N, np.float32)
    np.add.at(deg_w, dst, ew)
    deg_w += 1.0  # self-loop weight
    deg_1 = (np.bincount(dst, minlength=N) + 1).astype(np.float32)
    dinw = np.zeros(NPAD, np.float32)
    din1 = np.zeros(NPAD, np.float32)
    dinw[:N] = 1.0 / np.sqrt(deg_w)
    din1[:N] = 1.0 / np.sqrt(deg_1)

    core = dst // SH
    t_all = (dst % SH) // 128
    dloc = (dst % 128).astype(np.float32)
    q_all = src // (2 * SH)            # core-pair subtable index
    sub_row = (src % (2 * SH)).astype(np.int64)

    # per (core, q, t) counts -> shared chunk structure K[q, t] (>=1 so the
    # q0 self-loop matmul and the q3 finalize exist for every t)
    cell = (core * NQ + q_all) * NBLK + t_all
    cnt = np.bincount(cell, minlength=NCORES * NQ * NBLK).reshape(NCORES, NQ, NBLK)
    K = np.maximum(1, np.ceil(cnt.max(axis=0) / 128).astype(np.int64))  # [NQ, NBLK]
    base = np.zeros((NQ, NBLK), np.int64)
    for q in range(NQ):
        base[q] = np.cumsum(np.concatenate([[0], K[q][:-1]]))
    C = K.sum(axis=1)                                  # chunks per q-stream
    G = [int(math.ceil(int(c) / CPG)) for c in C]

    # chunkmap[q][chunk_pos] = (t, k, K[q,t]) or None
    chunkmap = []
    for q in range(NQ):
        cm = [None] * (G[q] * CPG)
        for t in range(NBLK):
            for k in range(int(K[q, t])):
                cm[int(base[q, t]) + k] = (t, k, int(K[q, t]))
        chunkmap.append(cm)

    # per-core slot fill
    per_core = []
    # innermost sub_row sort -> ascending gather addresses within each cell
    # (HBM row-buffer locality for the dma_gather descriptors)
    order = np.lexsort((sub_row, t_all, q_all, core))
    osub, oq, ot, odl, oew, ocore = (
        sub_row[order], q_all[order], t_all[order],
        dloc[order], ew[order], core[order],
    )
    cstart = np.searchsorted(ocore, np.arange(NCORES + 1))
    for d in range(NCORES):
        lo, hi = cstart[d], cstart[d + 1]
        dq, dt = oq[lo:hi], ot[lo:hi]
        dsub, ddl, dew = osub[lo:hi], odl[lo:hi], oew[lo:hi]
        cellk = dq * NBLK + dt
        cello = np.searchsorted(cellk, np.arange(NQ * NBLK + 1))
        srcw, dlv, ewv = [], [], []
        for q in range(NQ):
            nslot = G[q] * SLOTS
            sl = np.zeros(nslot, np.int64)
            dl = np.full(nslot, SENT, np.float32)
            wv = np.zeros(nslot, np.float32)
            for t in range(NBLK):
                a, b = cello[q * NBLK + t], cello[q * NBLK + t + 1]
                n = b - a
                p0 = int(base[q, t]) * 128
                sl[p0:p0 + n] = dsub[a:b]
                dl[p0:p0 + n] = ddl[a:b]
                wv[p0:p0 + n] = dew[a:b]
            # idx: [G*SLOTS] -> [128, G, SLOTS//16] (16-wrap, replicated x8)
            g = sl.astype(np.int16).reshape(G[q], SLOTS // 16, 16)
            g = np.swapaxes(g, 1, 2)                       # [G, 16, S//16]
            g = np.tile(g, (1, 8, 1))                      # [G, 128, S//16]
            srcw.append(np.ascontiguousarray(g.transpose(1, 0, 2)))
            # dl/ew: [G*SLOTS] -> [128, G, CPG]
            dlv.append(np.ascontiguousarray(
                dl.astype(np.float16).reshape(G[q], CPG, 128).transpose(2, 0, 1)))
            ewv.append(np.ascontiguousarray(
                wv.astype(np.float16).reshape(G[q], CPG, 128).transpose(2, 0, 1)))
        per_core.append((srcw, dlv, ewv))

    return chunkmap, G, dinw, din1, per_core


def _build(chunkmap, G):
    f32 = mybir.dt.float32
    f16 = mybir.dt.float16
    nc = bacc.Bacc(None, target_bir_lowering=False, num_swdge_queues=4, num_devices=NCORES)

    xs_d = nc.dram_tensor("xs", [128, NBLK, FIN], f32, kind="ExternalInput")
    srcw_d = [nc.dram_tensor(f"srcw{q}", [128, G[q], SLOTS // 16], mybir.dt.int16, kind="ExternalInput") for q in range(NQ)]
    dl_d = [nc.dram_tensor(f"dl{q}", [128, G[q], CPG], f16, kind="ExternalInput") for q in range(NQ)]
    ew_d = [nc.dram_tensor(f"ew{q}", [128, G[q], CPG], f16, kind="ExternalInput") for q in range(NQ)]
    dinw_d = nc.dram_tensor("dinw", [128, NBLK], f32, kind="ExternalInput")
    din1_d = nc.dram_tensor("din1", [128, NBLK], f32, kind="ExternalInput")
    w1_d = nc.dram_tensor("W1", [FIN, HID], f16, kind="ExternalInput")
    w2_d = nc.dram_tensor("W2", [HID, HID], f16, kind="ExternalInput")
    wmls_d = nc.dram_tensor("Wmls", [HID, 2 * OUT], f16, kind="ExternalInput")
    b1_d = nc.dram_tensor("b1", [HID, 1], f32, kind="ExternalInput")
    b2_d = nc.dram_tensor("b2", [HID, 1], f32, kind="ExternalInput")
    bmls_d = nc.dram_tensor("bmls", [2 * OUT, 1], f32, kind="ExternalInput")
    iota_d = nc.dram_tensor("iota", [128, SLOTS], f16, kind="ExternalInput")
    ident_d = nc.dram_tensor("ident", [128, 128], f16, kind="ExternalInput")
    out_d = nc.dram_tensor("out", [SH, 2 * OUT], f32, kind="ExternalOutput")

    ag_in = [nc.dram_tensor(f"ag_in{l}", [SH, TF], f16) for l in range(3)]
    tables = [nc.dram_tensor(f"table{l}", [NPAD, TF], f16, addr_space="Shared") for l in range(3)]

    with tile.TileContext(nc) as tc:
        with (
            tc.tile_pool(name="const", bufs=1) as kpool,
            tc.tile_pool(name="g", bufs=10) as gpool,
            tc.tile_pool(name="b", bufs=8) as bpool,
            tc.tile_pool(name="gs", bufs=6) as gspool,
            tc.tile_pool(name="tmp", bufs=4) as tpool,
            tc.tile_pool(name="pagg", bufs=4, space="PSUM") as pagg,
            tc.tile_pool(name="ptr1", bufs=1, space="PSUM") as ptr1,
            tc.tile_pool(name="pmm", bufs=2, space="PSUM") as pmm,
            tc.tile_pool(name="ptr2", bufs=1, space="PSUM") as ptr2,
        ):
            nc.gpsimd.load_library(mlp)

            iota_t = kpool.tile([128, SLOTS], f16)
            nc.sync.dma_start(iota_t[:], iota_d[:])
            ident_t = kpool.tile([128, 128], f16)
            nc.sync.dma_start(ident_t[:], ident_d[:])
            w1_t = kpool.tile([FIN, HID], f16)
            nc.sync.dma_start(w1_t[:], w1_d[:])
            w2_t = kpool.tile([HID, HID], f16)
            nc.sync.dma_start(w2_t[:], w2_d[:])
            wmls_t = kpool.tile([HID, 2 * OUT], f16)
            nc.sync.dma_start(wmls_t[:], wmls_d[:])
            b1_t = kpool.tile([HID, 1], f32)
            nc.sync.dma_start(b1_t[:], b1_d[:])
            b2_t = kpool.tile([HID, 1], f32)
            nc.sync.dma_start(b2_t[:], b2_d[:])
            bmls_t = kpool.tile([2 * OUT, 1], f32)
            nc.sync.dma_start(bmls_t[:], bmls_d[:])
            dinw_t = kpool.tile([128, NBLK], f32)
            nc.sync.dma_start(dinw_t[:], dinw_d[:])
            din1_t = kpool.tile([128, NBLK], f32)
            nc.sync.dma_start(din1_t[:], din1_d[:])

            idx_t = [kpool.tile([128, G[q], SLOTS // 16], mybir.dt.int16, name=f"idx_t{q}") for q in range(NQ)]
            dl_t = [kpool.tile([128, G[q], CPG], f16, name=f"dl_t{q}") for q in range(NQ)]
            ew_t = [kpool.tile([128, G[q], CPG], f16, name=f"ew_t{q}") for q in range(NQ)]
            for q in range(NQ):
                nc.sync.dma_start(idx_t[q][:], srcw_d[q][:])
                nc.sync.dma_start(dl_t[q][:], dl_d[q][:])
                nc.sync.dma_start(ew_t[q][:], ew_d[q][:])

            st_a = kpool.tile([128, NBLK, HID], f16)
            st_b = kpool.tile([128, NBLK, HID], f16)
            aggsb = kpool.tile([128, NBLK, HID], f32)
            agq = [kpool.tile([128, QBLK, TF], f16, name=f"agq{i}") for i in range(2)]
            nc.vector.memset(agq[0][:], 0.0)
            nc.vector.memset(agq[1][:], 0.0)
            outq = [kpool.tile([128, QBLK, 2 * OUT], f32, name=f"outq{i}") for i in range(2)]

            iota_r = iota_t[:].rearrange("p (j v) -> p j v", j=CPG)
            gq = [0]

            def fire_ag(l, qd):
                nc.sync.dma_start(
                    ag_in[l][qd * QROWS:(qd + 1) * QROWS, :].rearrange("(t p) f -> p t f", p=128),
                    agq[qd % 2][:],
                )
                if qd == NQ - 1:
                    nc.gpsimd.collective_compute(
                        "AllGather", mybir.AluOpType.bypass,
                        replica_groups=[list(range(NCORES))],
                        ins=[ag_in[l][:]], outs=[tables[l][:]],
                    )

            def store_h(l, t, pt2):
                """pt2 [128 rows, F] psum -> st/agq (pre-scaled) or outq; fire AG/out."""
                qd, tt = t // QBLK, t % QBLK
                if l == 2:
                    nc.scalar.copy(outq[qd % 2][:, tt, :], pt2[:])
                    if tt == QBLK - 1:
                        nc.sync.dma_start(
                            out_d[qd * QROWS:(qd + 1) * QROWS, :].rearrange("(t p) f -> p t f", p=128),
                            outq[qd % 2][:],
                        )
                    return
                st_dst = st_b if l == 0 else st_a
                pre = dinw_t if l == 0 else din1_t
                nc.scalar.mul(st_dst[:, t, :], pt2[:], pre[:, t:t + 1])
                nc.scalar.copy(agq[qd % 2][:, tt, :HID], st_dst[:, t, :])
                if tt == QBLK - 1:
                    fire_ag(l + 1, qd)

            def project(l, t, aggf):
                """aggf [128 rows, HID] f16 -> h rows via W/bias/act -> store_h."""
                pt1 = ptr1.tile([HID, 128], f16, tag="pt1")
                nc.tensor.transpose(pt1[:], aggf[:], ident_t[:])
                if l == 0:
                    hb = tpool.tile([HID, 128], f16, tag="hb")
                    nc.scalar.activation(hb[:], pt1[:], mybir.ActivationFunctionType.Relu, bias=b1_t[:])
                else:
                    aggT = tpool.tile([HID, 128], f16, tag="aggT")
                    nc.scalar.copy(aggT[:], pt1[:])
                    w, b, fo = (w2_t, b2_t, HID) if l == 1 else (wmls_t, bmls_t, 2 * OUT)
                    pm = pmm.tile([fo, 128], f32, tag="pm")
                    nc.tensor.matmul(pm[:], lhsT=w[:], rhs=aggT[:], start=True, stop=True)
                    hb = tpool.tile([fo, 128], f16, tag="hb")
                    nc.vector.tensor_scalar_add(hb[:], pm[:], b[:])
                fo = hb.shape[0]
                pt2 = ptr2.tile([128, fo], f16, tag="pt2")
                nc.tensor.transpose(pt2[:], hb[:], ident_t[:fo, :fo])
                store_h(l, t, pt2)

            # ---- pre-phase: table0 rows = (x @ W1) * dinw, by quarter ----
            for t in range(NBLK):
                xt = tpool.tile([128, FIN], f32, tag="xt")
                nc.sync.dma_start(xt[:], xs_d[:, t, :])
                xth = tpool.tile([128, FIN], f16, tag="xth")
                nc.scalar.copy(xth[:], xt[:])
                px = ptr1.tile([FIN, 128], f16, tag="pt1")
                nc.tensor.transpose(px[:], xth[:], ident_t[:])
                xT = tpool.tile([FIN, 128], f16, tag="xT")
                nc.scalar.copy(xT[:], px[:])
                pm = pmm.tile([HID, 128], f32, tag="pm")
                nc.tensor.matmul(pm[:], lhsT=w1_t[:], rhs=xT[:], start=True, stop=True)
                h0 = tpool.tile([HID, 128], f16, tag="hb")
                nc.scalar.copy(h0[:], pm[:])
                pt2 = ptr2.tile([128, HID], f16, tag="pt2")
                nc.tensor.transpose(pt2[:], h0[:], ident_t[:HID, :HID])
                qd, tt = t // QBLK, t % QBLK
                nc.scalar.mul(st_a[:, t, :], pt2[:], dinw_t[:, t:t + 1])
                nc.scalar.copy(agq[qd % 2][:, tt, :HID], st_a[:, t, :])
                if tt == QBLK - 1:
                    fire_ag(0, qd)

            # ---- layers ----
            for l in range(3):
                st_src = st_a if l != 1 else st_b
                evscale = dinw_t if l < 2 else din1_t
                cur_ps = {}
                for q in range(NQ):
                    cm = chunkmap[q]
                    for g in range(G[q]):
                        gt = gpool.tile([128, CPG, TF], f16, tag="gt")
                        # issue as f32 over bit-paired f16 values: halves the
                        # per-descriptor element count the SDMA datapath walks
                        nc.gpsimd.dma_gather(
                            gt[:].bitcast(f32),
                            tables[l][q * SUBROWS:(q + 1) * SUBROWS, :].bitcast(f32),
                            idx_t[q][:, g, :],
                            SLOTS, SLOTS, TF // 2, queue_num=gq[0] % 4,
                        )
                        gq[0] += 1
                        if l < 2:
                            gs = gspool.tile([128, CPG, HID], f16, tag="gs")
                            nc.vector.tensor_tensor(
                                out=gs[:], in0=gt[:, :, :HID],
                                in1=ew_t[q][:, g, :].to_broadcast([128, CPG, HID]),
                                op=mybir.AluOpType.mult,
                            )
                        bt = bpool.tile([128, CPG, 128], f16, tag="b")
                        nc.vector.tensor_tensor(
                            out=bt[:], in0=iota_r,
                            in1=dl_t[q][:, g, :].to_broadcast([128, CPG, 128]),
                            op=mybir.AluOpType.is_equal,
                        )
                        for j in range(CPG):
                            ent = cm[g * CPG + j]
                            if ent is None:
                                continue
                            t, k, Kt = ent
                            rhs = gs[:, j, :] if l < 2 else gt[:, j, :HID]
                            last = k == Kt - 1
                            if k == 0:
                                ps = pagg.tile([128, HID], f32, tag="pagg")
                                cur_ps[t] = ps
                                if q == 0:
                                    nc.tensor.matmul(ps[:], lhsT=ident_t[:], rhs=st_src[:, t, :], start=True, stop=False)
                                    nc.tensor.matmul(ps[:], lhsT=bt[:, j, :], rhs=rhs, start=False, stop=last)
                                else:
                                    nc.tensor.matmul(ps[:], lhsT=bt[:, j, :], rhs=rhs, start=True, stop=last)
                            else:
                                ps = cur_ps[t]
                                nc.tensor.matmul(ps[:], lhsT=bt[:, j, :], rhs=rhs, start=False, stop=last)
                            if last:
                                if q == 0:
                                    nc.vector.tensor_copy(aggsb[:, t, :], ps[:])
                                else:
                                    nc.vector.tensor_tensor(
                                        out=aggsb[:, t, :], in0=aggsb[:, t, :], in1=ps[:],
                                        op=mybir.AluOpType.add,
                                    )
                                    if q == NQ - 1:
                                        aggf = tpool.tile([128, HID], f16, tag="aggf")
                                        nc.scalar.mul(aggf[:], aggsb[:, t, :], evscale[:, t:t + 1])
                                        project(l, t, aggf)

    # Tile round-robins Pool-DMA completion sems over 8 DMASW lanes without
    # queue awareness, but each sem is hardware-locked to the first SWDGE
    # queue that increments it. Rewrite each gather's queue to lane % 4 so
    # every lane's sem is only ever incremented from one queue.
    for fn in nc.m.functions:
        for blk in fn.blocks:
            for ins in blk.instructions:
                if isinstance(ins, mybir.InstDMAGatherAnt) and ins.sync_info:
                    for u in ins.sync_info.on_update:
                        name = getattr(u, "ant_name", "") or ""
                        if name.startswith("DMASW"):
                            ins.queue_num = int(name[5:].split("_")[0]) % 4
                            break

    nc.compile()
    return nc


def _run(inputs, trace=False):
    x = np.asarray(inputs["x"], np.float32)
    chunkmap, G, dinw, din1, per_core = _prep(
        np.asarray(inputs["edge_index"]), np.asarray(inputs["edge_weight"])
    )
    nc = _build(chunkmap, G)

    x_pad = np.zeros((NPAD, FIN), np.float32)
    x_pad[:N] = x
    wmls = np.concatenate(
        [np.asarray(inputs["Wmu"], np.float32), np.asarray(inputs["Wls"], np.float32)], axis=1
    )
    bmls = np.concatenate(
        [np.asarray(inputs["bmu"], np.float32), np.asarray(inputs["bls"], np.float32)]
    )
    iota = np.tile(np.arange(128, dtype=np.float16)[None, :], (128, CPG)).reshape(128, SLOTS)
    shared = {
        "W1": np.asarray(inputs["W1"], np.float32).astype(np.float16),
        "W2": np.asarray(inputs["W2"], np.float32).astype(np.float16),
        "Wmls": wmls.astype(np.float16),
        "b1": np.asarray(inputs["b1"], np.float32).reshape(HID, 1),
        "b2": np.asarray(inputs["b2"], np.float32).reshape(HID, 1),
        "bmls": bmls.astype(np.float32).reshape(2 * OUT, 1),
        "iota": iota,
        "ident": np.eye(128, dtype=np.float16),
    }
    in_maps = []
    for d in range(NCORES):
        srcw, dlv, ewv = per_core[d]
        m = dict(shared)
        m["xs"] = np.ascontiguousarray(
            x_pad[d * SH:(d + 1) * SH].reshape(NBLK, 128, FIN).transpose(1, 0, 2)
        )
        m["dinw"] = np.ascontiguousarray(dinw[d * SH:(d + 1) * SH].reshape(NBLK, 128).T)
        m["din1"] = np.ascontiguousarray(din1[d * SH:(d + 1) * SH].reshape(NBLK, 128).T)
        for q in range(NQ):
            m[f"srcw{q}"] = srcw[q]
            m[f"dl{q}"] = dlv[q]
            m[f"ew{q}"] = ewv[q]
        in_maps.append(m)

    res = run_bass_kernel_spmd(nc, in_maps, core_ids=list(range(NCORES)), trace=trace)
    full = np.concatenate([res.results[d]["out"] for d in range(NCORES)], axis=0)
    mu = full[:N, :OUT].copy()
    logstd = full[:N, OUT:].copy()
    return (mu, logstd), res


def kernel(**inputs):
    (mu, logstd), _ = _run(inputs, trace=False)
    return mu, logstd


# revision 15
# speedup vs baseline: 1.0274x; 1.0274x over previous
"""GCN VGAE encoder (nn_Encoder_25065429139538) on 8 Trainium2 NeuronCores.

Strategy (sharding_hint: shard nodes across cores, partition edges by dst,
replicate weights):
  - Nodes padded to 102400 = 8 x 12800; core d owns dst rows [d*SH, (d+1)*SH).
  - GCN norm dinv[src]*ew*dinv[dst] is factored: dinv[src] is pre-multiplied
    into the stored gather-table rows, dinv[dst] is applied when evicting the
    per-block accumulator from PSUM. Layers 1/2 then carry only the raw edge
    weight ew per edge; the mu/logstd layer needs no per-edge scalar at all
    (duplicate (src,dst) pairs stay as distinct slots, reproducing the
    reference's multiplicity semantics).
  - Tables are f16 [rows, 128] (features 0..63 real, 64..127 zero) so each
    dma_gather descriptor is the 256B minimum. Edges are bucketed by
    (src quarter q, dst 128-block t); each (q,t) cell is padded to whole
    128-slot chunks shared across cores. Per chunk, a one-hot matrix
    B[slot, dloc] (f16, built on DVE from cached dloc bytes) is the
    *stationary* matmul operand (128 cols -> fast weight load) and the
    gathered rows stream through, accumulating agg[dst, F] in PSUM.
  - Self-loops are an identity-weight matmul reading the core's own
    projected rows straight from SBUF (q0 phase, before any AllGather).
  - Per-edge metadata (gather indices, dloc, ew) is SBUF-resident for the
    whole kernel, loaded once in a few large DMAs.
  - h is AllGather'd between layers in 4 quarter-collectives, each fired as
    soon as its quarter of the projection completes, so the next layer's
    gathers overlap the exchange. mu/logstd share one aggregation pass.
"""

import math

import numpy as np

import concourse.bass as bass
import concourse.bacc as bacc
import concourse.mybir as mybir
import concourse.tile as tile
from concourse.bass_utils import run_bass_kernel_spmd
from concourse.library_config import mlp

# ---- problem constants (hardcoded per contract) ----
N = 100000
FIN, HID, OUT = 128, 64, 32
NCORES = 8

# ---- layout constants ----
SH = 12800            # rows per core
NPAD = SH * NCORES    # 102400
NBLK = SH // 128      # 100 dst blocks per core
NQ = 4                # src quarters
QBLK = NBLK // NQ     # 25 dst blocks per quarter
QROWS = SH // NQ      # 3200 rows per quarter per core
SUBROWS = QROWS * NCORES  # 25600 rows per quarter subtable
TF = 128              # padded table feature dim (f16 -> 256B rows)
SLOTS = 1024          # gather slots per dma_gather instruction
CPG = SLOTS // 128    # chunks per gather group = 32
SENT = 255.0          # dloc sentinel for padding slots


def _prep(edge_index, edge_weight):
    """Host-side edge partitioning."""
    src = np.asarray(edge_index[0], dtype=np.int64)
    dst = np.asarray(edge_index[1], dtype=np.int64)
    ew = np.asarray(edge_weight, dtype=np.float32)

    deg_w = np.zeros(N, np.float32)
    np.add.at(deg_w, dst, ew)
    deg_w += 1.0  # self-loop weight
    deg_1 = (np.bincount(dst, minlength=N) + 1).astype(np.float32)
    dinw = np.zeros(NPAD, np.float32)
    din1 = np.zeros(NPAD, np.float32)
    dinw[:N] = 1.0 / np.sqrt(deg_w)
    din1[:N] = 1.0 / np.sqrt(deg_1)

    core = dst // SH
    t_all = (dst % SH) // 128
    dloc = (dst % 128).astype(np.float32)
    q_all = src // (2 * SH)            # core-pair subtable index
    sub_row = (src % (2 * SH)).astype(np.int64)

    # per (core, q, t) counts -> shared chunk structure K[q, t] (>=1 so the
    # q0 self-loop matmul and the q3 finalize exist for every t)
    cell = (core * NQ + q_all) * NBLK + t_all
    cnt = np.bincount(cell, minlength=NCORES * NQ * NBLK).reshape(NCORES, NQ, NBLK)
    K = np.maximum(1, np.ceil(cnt.max(axis=0) / 128).astype(np.int64))  # [NQ, NBLK]
    base = np.zeros((NQ, NBLK), np.int64)
    for q in range(NQ):
        base[q] = np.cumsum(np.concatenate([[0], K[q][:-1]]))
    C = K.sum(axis=1)                                  # chunks per q-stream
    G = [int(math.ceil(int(c) / CPG)) for c in C]

    # chunkmap[q][chunk_pos] = (t, k, K[q,t]) or None
    chunkmap = []
    for q in range(NQ):
        cm = [None] * (G[q] * CPG)
        for t in range(NBLK):
            for k in range(int(K[q, t])):
                cm[int(base[q, t]) + k] = (t, k, int(K[q, t]))
        chunkmap.append(cm)

    # per-core slot fill
    per_core = []
    # innermost sub_row sort -> ascending gather addresses within each cell
    # (HBM row-buffer locality for the dma_gather descriptors)
    order = np.lexsort((sub_row, t_all, q_all, core))
    osub, oq, ot, odl, oew, ocore = (
        sub_row[order], q_all[order], t_all[order],
        dloc[order], ew[order], core[order],
    )
    cstart = np.searchsorted(ocore, np.arange(NCORES + 1))
    for d in range(NCORES):
        lo, hi = cstart[d], cstart[d + 1]
        dq, dt = oq[lo:hi], ot[lo:hi]
        dsub, ddl, dew = osub[lo:hi], odl[lo:hi], oew[lo:hi]
        cellk = dq * NBLK + dt
        cello = np.searchsorted(cellk, np.arange(NQ * NBLK + 1))
        srcw, dlv, ewv = [], [], []
        for q in range(NQ):
            nslot = G[q] * SLOTS
            sl = np.zeros(nslot, np.int64)
            dl = np.full(nslot, SENT, np.float32)
            wv = np.zeros(nslot, np.float32)
            for t in range(NBLK):
                a, b = cello[q * NBLK + t], cello[q * NBLK + t + 1]
                n = b - a
                p0 = int(base[q, t]) * 128
                sl[p0:p0 + n] = dsub[a:b]
                dl[p0:p0 + n] = ddl[a:b]
                wv[p0:p0 + n] = dew[a:b]
            # idx: [G*SLOTS] -> [128, G, SLOTS//16] (16-wrap, replicated x8)
            g = sl.astype(np.int16).reshape(G[q], SLOTS // 16, 16)
            g = np.swapaxes(g, 1, 2)                       # [G, 16, S//16]
            g = np.tile(g, (1, 8, 1))                      # [G, 128, S//16]
            srcw.append(np.ascontiguousarray(g.transpose(1, 0, 2)))
            # dl/ew: [G*SLOTS] -> [128, G, CPG]
            dlv.append(np.ascontiguousarray(
                dl.astype(np.float16).reshape(G[q], CPG, 128).transpose(2, 0, 1)))
            ewv.append(np.ascontiguousarray(
                wv.astype(np.float16).reshape(G[q], CPG, 128).transpose(2, 0, 1)))
        per_core.append((srcw, dlv, ewv))

    return chunkmap, G, dinw, din1, per_core


def _build(chunkmap, G):
    f32 = mybir.dt.float32
    f16 = mybir.dt.float16
    nc = bacc.Bacc(None, target_bir_lowering=False, num_swdge_queues=4, num_devices=NCORES)

    xs_d = nc.dram_tensor("xs", [128, NBLK, FIN], f32, kind="ExternalInput")
    srcw_d = [nc.dram_tensor(f"srcw{q}", [128, G[q], SLOTS // 16], mybir.dt.int16, kind="ExternalInput") for q in range(NQ)]
    dl_d = [nc.dram_tensor(f"dl{q}", [128, G[q], CPG], f16, kind="ExternalInput") for q in range(NQ)]
    ew_d = [nc.dram_tensor(f"ew{q}", [128, G[q], CPG], f16, kind="ExternalInput") for q in range(NQ)]
    dinw_d = nc.dram_tensor("dinw", [128, NBLK], f32, kind="ExternalInput")
    din1_d = nc.dram_tensor("din1", [128, NBLK], f32, kind="ExternalInput")
    w1_d = nc.dram_tensor("W1", [FIN, HID], f16, kind="ExternalInput")
    w2_d = nc.dram_tensor("W2", [HID + 1, HID], f16, kind="ExternalInput")
    wmls_d = nc.dram_tensor("Wmls", [HID + 1, 2 * OUT], f16, kind="ExternalInput")
    b1_d = nc.dram_tensor("b1", [HID, 1], f32, kind="ExternalInput")
    iota_d = nc.dram_tensor("iota", [128, SLOTS], f16, kind="ExternalInput")
    ident_d = nc.dram_tensor("ident", [128, 128], f16, kind="ExternalInput")
    out_d = nc.dram_tensor("out", [SH, 2 * OUT], f32, kind="ExternalOutput")

    ag_in = [nc.dram_tensor(f"ag_in{l}", [SH, TF], f16) for l in range(3)]
    tables = [nc.dram_tensor(f"table{l}", [NPAD, TF], f16, addr_space="Shared") for l in range(3)]

    with tile.TileContext(nc) as tc:
        with (
            tc.tile_pool(name="const", bufs=1) as kpool,
            tc.tile_pool(name="g", bufs=10) as gpool,
            tc.tile_pool(name="b", bufs=8) as bpool,
            tc.tile_pool(name="gs", bufs=6) as gspool,
            tc.tile_pool(name="tmp", bufs=4) as tpool,
            tc.tile_pool(name="pagg", bufs=4, space="PSUM") as pagg,
            tc.tile_pool(name="ptr1", bufs=1, space="PSUM") as ptr1,
            tc.tile_pool(name="pmm", bufs=2, space="PSUM") as pmm,
            tc.tile_pool(name="ptr2", bufs=1, space="PSUM") as ptr2,
        ):
            nc.gpsimd.load_library(mlp)

            iota_t = kpool.tile([128, SLOTS], f16)
            nc.sync.dma_start(iota_t[:], iota_d[:])
            ident_t = kpool.tile([128, 128], f16)
            nc.sync.dma_start(ident_t[:], ident_d[:])
            w1_t = kpool.tile([FIN, HID], f16)
            nc.sync.dma_start(w1_t[:], w1_d[:])
            w2_t = kpool.tile([HID + 1, HID], f16)
            nc.sync.dma_start(w2_t[:], w2_d[:])
            wmls_t = kpool.tile([HID + 1, 2 * OUT], f16)
            nc.sync.dma_start(wmls_t[:], wmls_d[:])
            b1_t = kpool.tile([HID, 1], f32)
            nc.sync.dma_start(b1_t[:], b1_d[:])
            dinw_t = kpool.tile([128, NBLK], f32)
            nc.sync.dma_start(dinw_t[:], dinw_d[:])
            din1_t = kpool.tile([128, NBLK], f32)
            nc.sync.dma_start(din1_t[:], din1_d[:])

            idx_t = [kpool.tile([128, G[q], SLOTS // 16], mybir.dt.int16, name=f"idx_t{q}") for q in range(NQ)]
            dl_t = [kpool.tile([128, G[q], CPG], f16, name=f"dl_t{q}") for q in range(NQ)]
            ew_t = [kpool.tile([128, G[q], CPG], f16, name=f"ew_t{q}") for q in range(NQ)]
            for q in range(NQ):
                nc.sync.dma_start(idx_t[q][:], srcw_d[q][:])
                nc.sync.dma_start(dl_t[q][:], dl_d[q][:])
                nc.sync.dma_start(ew_t[q][:], ew_d[q][:])

            st_a = kpool.tile([128, NBLK, HID], f16)
            st_b = kpool.tile([128, NBLK, HID], f16)
            aggsb = kpool.tile([128, NBLK, HID], f16)
            agq = [kpool.tile([128, QBLK, TF], f16, name=f"agq{i}") for i in range(2)]
            nc.vector.memset(agq[0][:], 0.0)
            nc.vector.memset(agq[1][:], 0.0)
            outq = [kpool.tile([128, QBLK, 2 * OUT], f32, name=f"outq{i}") for i in range(2)]
            aggT_t = [kpool.tile([HID + 1, 128], f16, name=f"aggT{i}") for i in range(4)]
            for i in range(4):
                nc.vector.memset(aggT_t[i][HID:HID + 1, :], 1.0)

            iota_r = iota_t[:].rearrange("p (j v) -> p j v", j=CPG)
            gq = [0]

            def fire_ag(l, qd):
                nc.sync.dma_start(
                    ag_in[l][qd * QROWS:(qd + 1) * QROWS, :].rearrange("(t p) f -> p t f", p=128),
                    agq[qd % 2][:],
                )
                if qd == NQ - 1:
                    nc.gpsimd.collective_compute(
                        "AllGather", mybir.AluOpType.bypass,
                        replica_groups=[list(range(NCORES))],
                        ins=[ag_in[l][:]], outs=[tables[l][:]],
                    )

            def store_h(l, t, pt2):
                """pt2 [128 rows, F] psum -> st/agq (pre-scaled) or outq; fire AG/out."""
                qd, tt = t // QBLK, t % QBLK
                if l == 2:
                    nc.scalar.copy(outq[qd % 2][:, tt, :], pt2[:])
                    if tt == QBLK - 1:
                        nc.sync.dma_start(
                            out_d[qd * QROWS:(qd + 1) * QROWS, :].rearrange("(t p) f -> p t f", p=128),
                            outq[qd % 2][:],
                        )
                    return
                st_dst = st_b if l == 0 else st_a
                pre = dinw_t if l == 0 else din1_t
                nc.scalar.mul(st_dst[:, t, :], pt2[:], pre[:, t:t + 1])
                nc.scalar.copy(agq[qd % 2][:, tt, :HID], st_dst[:, t, :])
                if tt == QBLK - 1:
                    fire_ag(l + 1, qd)

            def project(l, t, aggf):
                """aggf [128 rows, HID] f16 -> h rows via W/bias/act -> store_h."""
                pt1 = ptr1.tile([HID, 128], f16, tag="pt1")
                nc.tensor.transpose(pt1[:], aggf[:], ident_t[:])
                if l == 0:
                    hb = tpool.tile([HID, 128], f16, tag="hb")
                    nc.scalar.activation(hb[:], pt1[:], mybir.ActivationFunctionType.Relu, bias=b1_t[:])
                else:
                    aggT = aggT_t[t % 4]
                    nc.scalar.copy(aggT[:HID, :], pt1[:])
                    w, fo = (w2_t, HID) if l == 1 else (wmls_t, 2 * OUT)
                    pm = pmm.tile([fo, 128], f32, tag="pm")
                    nc.tensor.matmul(pm[:], lhsT=w[:], rhs=aggT[:], start=True, stop=True)
                    hb = tpool.tile([fo, 128], f16, tag="hb")
                    nc.scalar.copy(hb[:], pm[:])
                fo = hb.shape[0]
                pt2 = ptr2.tile([128, fo], f16, tag="pt2")
                nc.tensor.transpose(pt2[:], hb[:], ident_t[:fo, :fo])
                store_h(l, t, pt2)

            # ---- pre-phase: table0 rows = (x @ W1) * dinw, by quarter ----
            for t in range(NBLK):
                xt = tpool.tile([128, FIN], f32, tag="xt")
                nc.sync.dma_start(xt[:], xs_d[:, t, :])
                xth = tpool.tile([128, FIN], f16, tag="xth")
                nc.scalar.copy(xth[:], xt[:])
                px = ptr1.tile([FIN, 128], f16, tag="pt1")
                nc.tensor.transpose(px[:], xth[:], ident_t[:])
                xT = tpool.tile([FIN, 128], f16, tag="xT")
                nc.scalar.copy(xT[:], px[:])
                pm = pmm.tile([HID, 128], f32, tag="pm")
                nc.tensor.matmul(pm[:], lhsT=w1_t[:], rhs=xT[:], start=True, stop=True)
                h0 = tpool.tile([HID, 128], f16, tag="hb")
                nc.scalar.copy(h0[:], pm[:])
                pt2 = ptr2.tile([128, HID], f16, tag="pt2")
                nc.tensor.transpose(pt2[:], h0[:], ident_t[:HID, :HID])
                qd, tt = t // QBLK, t % QBLK
                nc.scalar.mul(st_a[:, t, :], pt2[:], dinw_t[:, t:t + 1])
                nc.scalar.copy(agq[qd % 2][:, tt, :HID], st_a[:, t, :])
                if tt == QBLK - 1:
                    fire_ag(0, qd)

            # ---- layers ----
            for l in range(3):
                st_src = st_a if l != 1 else st_b
                evscale = dinw_t if l < 2 else din1_t
                cur_ps = {}
                for q in range(NQ):
                    cm = chunkmap[q]
                    for g in range(G[q]):
                        gt = gpool.tile([128, CPG, TF], f16, tag="gt")
                        # issue as f32 over bit-paired f16 values: halves the
                        # per-descriptor element count the SDMA datapath walks
                        nc.gpsimd.dma_gather(
                            gt[:].bitcast(f32),
                            tables[l][q * SUBROWS:(q + 1) * SUBROWS, :].bitcast(f32),
                            idx_t[q][:, g, :],
                            SLOTS, SLOTS, TF // 2, queue_num=gq[0] % 4,
                        )
                        gq[0] += 1
                        if l < 2:
                            gs = gspool.tile([128, CPG, HID], f16, tag="gs")
                            nc.vector.tensor_tensor(
                                out=gs[:], in0=gt[:, :, :HID],
                                in1=ew_t[q][:, g, :].to_broadcast([128, CPG, HID]),
                                op=mybir.AluOpType.mult,
                            )
                        bt = bpool.tile([128, CPG, 128], f16, tag="b")
                        nc.vector.tensor_tensor(
                            out=bt[:], in0=iota_r,
                            in1=dl_t[q][:, g, :].to_broadcast([128, CPG, 128]),
                            op=mybir.AluOpType.is_equal,
                        )
                        for j in range(CPG):
                            ent = cm[g * CPG + j]
                            if ent is None:
                                continue
                            t, k, Kt = ent
                            rhs = gs[:, j, :] if l < 2 else gt[:, j, :HID]
                            last = k == Kt - 1
                            if k == 0:
                                ps = pagg.tile([128, HID], f32, tag="pagg")
                                cur_ps[t] = ps
                                # cross-quarter partials live in aggsb (f16);
                                # reload via identity matmul so no DVE op ever
                                # depends on the PE (head-of-line blocking)
                                seed = st_src[:, t, :] if q == 0 else aggsb[:, t, :]
                                nc.tensor.matmul(ps[:], lhsT=ident_t[:], rhs=seed, start=True, stop=False)
                                nc.tensor.matmul(ps[:], lhsT=bt[:, j, :], rhs=rhs, start=False, stop=last)
                            else:
                                ps = cur_ps[t]
                                nc.tensor.matmul(ps[:], lhsT=bt[:, j, :], rhs=rhs, start=False, stop=last)
                            if last:
                                if q < NQ - 1:
                                    nc.scalar.copy(aggsb[:, t, :], ps[:])
                                else:
                                    aggf = tpool.tile([128, HID], f16, tag="aggf")
                                    nc.scalar.mul(aggf[:], ps[:], evscale[:, t:t + 1])
                                    project(l, t, aggf)

    # Tile round-robins Pool-DMA completion sems over 8 DMASW lanes without
    # queue awareness, but each sem is hardware-locked to the first SWDGE
    # queue that increments it. Rewrite each gather's queue to lane % 4 so
    # every lane's sem is only ever incremented from one queue.
    for fn in nc.m.functions:
        for blk in fn.blocks:
            for ins in blk.instructions:
                if isinstance(ins, mybir.InstDMAGatherAnt) and ins.sync_info:
                    for u in ins.sync_info.on_update:
                        name = getattr(u, "ant_name", "") or ""
                        if name.startswith("DMASW"):
                            ins.queue_num = int(name[5:].split("_")[0]) % 4
                            break

    nc.compile()
    return nc


def _run(inputs, trace=False):
    x = np.asarray(inputs["x"], np.float32)
    chunkmap, G, dinw, din1, per_core = _prep(
        np.asarray(inputs["edge_index"]), np.asarray(inputs["edge_weight"])
    )
    nc = _build(chunkmap, G)

    x_pad = np.zeros((NPAD, FIN), np.float32)
    x_pad[:N] = x
    wmls = np.concatenate(
        [np.asarray(inputs["Wmu"], np.float32), np.asarray(inputs["Wls"], np.float32)], axis=1
    )
    bmls = np.concatenate(
        [np.asarray(inputs["bmu"], np.float32), np.asarray(inputs["bls"], np.float32)]
    )
    iota = np.tile(np.arange(128, dtype=np.float16)[None, :], (128, CPG)).reshape(128, SLOTS)
    shared = {
        "W1": np.asarray(inputs["W1"], np.float32).astype(np.float16),
        "W2": np.vstack([np.asarray(inputs["W2"], np.float32),
                         np.asarray(inputs["b2"], np.float32).reshape(1, HID)]).astype(np.float16),
        "Wmls": np.vstack([wmls, bmls.reshape(1, 2 * OUT)]).astype(np.float16),
        "b1": np.asarray(inputs["b1"], np.float32).reshape(HID, 1),
        "iota": iota,
        "ident": np.eye(128, dtype=np.float16),
    }
    in_maps = []
    for d in range(NCORES):
        srcw, dlv, ewv = per_core[d]
        m = dict(shared)
        m["xs"] = np.ascontiguousarray(
            x_pad[d * SH:(d + 1) * SH].reshape(NBLK, 128, FIN).transpose(1, 0, 2)
        )
        m["dinw"] = np.ascontiguousarray(dinw[d * SH:(d + 1) * SH].reshape(NBLK, 128).T)
        m["din1"] = np.ascontiguousarray(din1[d * SH:(d + 1) * SH].reshape(NBLK, 128).T)
        for q in range(NQ):
            m[f"srcw{q}"] = srcw[q]
            m[f"dl{q}"] = dlv[q]
            m[f"ew{q}"] = ewv[q]
        in_maps.append(m)

    res = run_bass_kernel_spmd(nc, in_maps, core_ids=list(range(NCORES)), trace=trace)
    full = np.concatenate([res.results[d]["out"] for d in range(NCORES)], axis=0)
    mu = full[:N, :OUT].copy()
    logstd = full[:N, OUT:].copy()
    return (mu, logstd), res


def kernel(**inputs):
    (mu, logstd), _ = _run(inputs, trace=False)
    return mu, logstd
